# revision 5
# baseline (speedup 1.0000x reference)
"""LIF multicompartment refractory cell step on 8 Trainium2 NeuronCores.

Data-parallel over batch: each core handles B_LOC=512 of B=4096 rows.
On-device layout is transposed ([H, B_loc]) and fully host-preswizzled so
every DMA is a flat [128, X] transfer. The hidden/contraction dim sits on
SBUF partitions, so the three GEMMs need no on-device transposes:

  out1 = v @ (g_coupling + 0.9 I).T   (K=2048, f32r: feeds spike threshold)
  out2 = inp @ Wi.T + z @ Wr.T        (inp part bf16; z part fp8-e4m3
                                       DoubleRow in its own PSUM group)

Mixed precision keeps DMA (~50 MB/core) under the tensor roofline and
DoubleRow halves the z-GEMM's PE time; only the coupling GEMM (which the
spike threshold reads) stays f32r. State loads are fused per h-pair and
stores are split by readiness ([z|v], [rho], [i] per half) on the ACT
sequencer so producer waits never head-of-line-block the SP DMA queue.
The first h-pair's matmuls are emitted in k-quarters matched to the
hand-ordered head DMA stream, a junk-matmul prologue keeps the PE clock
warm, and the elementwise work is split DVE/Pool (rho chain on Pool) so
the DVE FIFO drains before the last matmul retires.
"""
import os
import numpy as np
import ml_dtypes

import concourse.bacc as bacc
import concourse.mybir as mybir
import concourse.tile as tile
from concourse import bass_utils

B, I, H = 4096, 2048, 2048
NCORES = 8
B_LOC = B // NCORES          # 512
HT = H // 128                # 16 h-tiles
HP = HT // 2                 # 8 h-pairs
KT1 = H // 128               # 16 k-tiles, coupling GEMM
KT2 = (I + H) // 128         # 32 k-tiles, i_new GEMM

BF16 = ml_dtypes.bfloat16
FP8 = ml_dtypes.float8_e4m3

_cache = {}


def build():
    nc = bacc.Bacc("TRN2", target_bir_lowering=False, debug=False,
                   num_devices=NCORES)
    f32 = mybir.dt.float32
    f32r = mybir.dt.float32r
    bf16 = mybir.dt.bfloat16
    A = mybir.AluOpType

    # activations, host layout [p, kt, b] flattened -> [128, KT1*B_LOC]
    vt_d = nc.dram_tensor("vt", [128, KT1 * B_LOC], f32r, kind="ExternalInput")
    fp8 = mybir.dt.float8e4
    zt_d = nc.dram_tensor("zt", [128, KT1 * B_LOC], fp8, kind="ExternalInput")
    xt_d = nc.dram_tensor("xt", [128, KT1 * B_LOC], bf16, kind="ExternalInput")
    # fused state stream: per h-pair [i (2*B_LOC) | rho (2*B_LOC)] in bf16
    st_d = nc.dram_tensor("st", [128, HP * 4 * B_LOC], bf16,
                          kind="ExternalInput")
    # weights pre-swizzled: [p, ht, kt, c] -> [128, HT*KT*128]
    w1_d = nc.dram_tensor("w1", [128, HT * KT1 * 128], f32r,
                          kind="ExternalInput")
    w2_d = nc.dram_tensor("w2", [128, HT * KT1 * 128], bf16,
                          kind="ExternalInput")
    w3_d = nc.dram_tensor("w3", [128, HT * KT1 * 128], fp8,
                          kind="ExternalInput")

    # fused outputs: per h-pair [z | v | i | rho], each 2*B_LOC, bf16
    out_d = nc.dram_tensor("out", [128, HP * 8 * B_LOC], bf16,
                           kind="ExternalOutput")

    with tile.TileContext(nc) as tc:
        with (
            tc.tile_pool(name="resid", bufs=1) as resid,
            tc.tile_pool(name="wpool", bufs=6) as wpool,
            tc.tile_pool(name="spool", bufs=2) as spool,
            tc.tile_pool(name="epool", bufs=2) as epool,
            tc.tile_pool(name="opool", bufs=2) as opool,
            tc.tile_pool(name="pspool", bufs=2, space="PSUM") as pspool,
            tc.tile_pool(name="warmp", bufs=1, space="PSUM") as warmp,
        ):
            vt_sb = resid.tile([128, KT1 * B_LOC], f32r)
            zt_sb = resid.tile([128, KT1, B_LOC], fp8)
            xt_sb = resid.tile([128, KT1 * B_LOC], bf16)

            # PE warm-up: ~tiny matmuls on junk SBUF keep the tensor engine
            # continuously busy from t=0 so the HAM clock is at full rate by
            # the time the first real (DMA-fed) matmul issues.
            wj = resid.tile([128, 192], bf16)
            nc.gpsimd.memset(wj[:], 0.0)
            psw = warmp.tile([128, 64], f32, name="psw")
            for _ in range(160):
                nc.tensor.matmul(psw[:], wj[:, :128], wj[:, 128:],
                                 start=True, stop=True)

            def load_resid(dst, src, chunks):
                n = KT1 // chunks
                for c in range(chunks):
                    cw = slice(c * n * B_LOC, (c + 1) * n * B_LOC)
                    nc.sync.dma_start(dst[:, cw], src[:, cw])

            def w1_load(h):
                w1 = wpool.tile([128, KT1 * 128], f32r, name="w1t")
                nc.sync.dma_start(
                    w1[:], w1_d[:, h * KT1 * 128:(h + 1) * KT1 * 128])
                return w1

            def w2_load(h):
                w2 = wpool.tile([128, KT1 * 128], bf16, name="w2t")
                nc.sync.dma_start(
                    w2[:], w2_d[:, h * KT1 * 128:(h + 1) * KT1 * 128])
                w3 = wpool.tile([128, KT1, 128], fp8, name="w3t")
                nc.sync.dma_start(
                    w3[:, :, :], w3_d[:, h * KT1 * 128:(h + 1) * KT1 * 128])
                return (w2, w3)

            def wc_load(h):
                return (w1_load(h), w2_load(h))

            def chunk_resid(dst, src, c, n):
                cw = slice(c * B_LOC, (c + n) * B_LOC)  # c, n in k-tiles
                nc.sync.dma_start(dst[:, cw], src[:, cw])

            def chunk_zt(c, n):
                cw = slice(c * B_LOC, (c + n) * B_LOC)
                nc.sync.dma_start(zt_sb[:, c:c + n, :], zt_d[:, cw])

            def st_load(hp):
                st = spool.tile([128, 4 * B_LOC], bf16, name="st")
                nc.sync.dma_start(
                    st[:], st_d[:, hp * 4 * B_LOC:(hp + 1) * 4 * B_LOC])
                return st

            def mm1(ps1, w1, kr):
                for k in kr:
                    nc.tensor.matmul(
                        ps1[:], w1[:, k * 128:(k + 1) * 128],
                        vt_sb[:, k * B_LOC:(k + 1) * B_LOC],
                        start=(k == 0), stop=(k == KT1 - 1))

            def mm2x(ps2, w23, kr):
                for k in kr:
                    nc.tensor.matmul(
                        ps2[:], w23[0][:, k * 128:(k + 1) * 128],
                        xt_sb[:, k * B_LOC:(k + 1) * B_LOC],
                        start=(k == 0), stop=(k == KT1 - 1))

            def mm2z(ps2z, w23, kr):
                for k in kr:
                    if k % 2:
                        continue
                    nc.tensor.matmul(
                        ps2z[:], w23[1][:, k:k + 2, :],
                        zt_sb[:, k:k + 2, :],
                        start=(k == 0), stop=(k == KT1 - 2),
                        perf_mode=mybir.MatmulPerfMode.DoubleRow)

            for hp in range(HP):
                pw = slice(2 * hp * B_LOC, (2 * hp + 2) * B_LOC)

                if hp == 0:
                    # head: DMA issue order = PE consumption order; h0/h1
                    # matmuls are emitted in k-quarters matched to arrivals
                    w1a = w1_load(0)
                    chunk_resid(vt_sb, vt_d, 0, 4)
                    w2a = w2_load(0)
                    chunk_resid(xt_sb, xt_d, 0, 4)
                    w1b = w1_load(1)
                    chunk_resid(vt_sb, vt_d, 4, 4)
                    chunk_resid(xt_sb, xt_d, 4, 4)
                    w2b = w2_load(1)
                    chunk_zt(0, 4)
                    st = st_load(0)
                    chunk_resid(vt_sb, vt_d, 8, 4)
                    chunk_resid(xt_sb, xt_d, 8, 4)
                    chunk_zt(4, 4)
                    chunk_resid(vt_sb, vt_d, 12, 4)
                    chunk_resid(xt_sb, xt_d, 12, 4)
                    wcs = [(w1a, w2a), (w1b, w2b)]
                    wnext = wc_load(2)              # prefetch hp1 weights
                    chunk_zt(8, 4)
                    wnext2 = wc_load(3)
                    chunk_zt(12, 4)

                    ps = [pspool.tile([128, B_LOC], f32, name="ps1")
                          for _ in range(2)]
                    ps2s = [pspool.tile([128, B_LOC], f32, name="ps2")
                            for _ in range(2)]
                    ps2zs = [pspool.tile([128, B_LOC], f32, name="ps2z")
                             for _ in range(2)]
                    q = [range(4 * j, 4 * j + 4) for j in range(8)]
                    mm1(ps[0], w1a[:], q[0])
                    mm2x(ps2s[0], w2a, q[0])
                    mm1(ps[1], w1b[:], q[0])
                    mm2x(ps2s[1], w2b, q[0])
                    for j in (1, 2, 3):
                        mm1(ps[0], w1a[:], q[j])
                        mm1(ps[1], w1b[:], q[j])
                        mm2x(ps2s[0], w2a, q[j])
                        mm2x(ps2s[1], w2b, q[j])
                        mm2z(ps2zs[0], w2a, q[j - 1])  # z-part
                        mm2z(ps2zs[1], w2b, q[j - 1])
                    mm2z(ps2zs[0], w2a, q[3])
                    mm2z(ps2zs[1], w2b, q[3])
                elif hp == 1:
                    st = st_load(1)
                    wcs = [wnext, wnext2]
                else:
                    st = st_load(hp)
                    wcs = [wc_load(2 * hp), wc_load(2 * hp + 1)]
                i2 = st[:, :2 * B_LOC]
                r2 = st[:, 2 * B_LOC:]

                # coupling GEMM for both halves first, so the elementwise
                # chain starts while the i_new GEMM still runs
                if hp != 0:
                    ps = []
                    for hh in range(2):
                        ps1 = pspool.tile([128, B_LOC], f32, name="ps1")
                        mm1(ps1, wcs[hh][0][:], range(KT1))
                        ps.append(ps1)

                v2 = vt_sb[:, pw].bitcast(f32)
                ot = opool.tile([128, 8 * B_LOC], bf16, name="ot")
                zo_t = ot[:, 0 * B_LOC:2 * B_LOC]
                vo_t = ot[:, 2 * B_LOC:4 * B_LOC]
                io_t = ot[:, 4 * B_LOC:6 * B_LOC]
                ro_t = ot[:, 6 * B_LOC:8 * B_LOC]

                # vdec = 0.1*i + ps1   (0.9*v is folded into w1's diagonal)
                vdec = epool.tile([128, 2 * B_LOC], f32, name="vdec")
                for hh in range(2):
                    hw = slice(hh * B_LOC, (hh + 1) * B_LOC)
                    nc.vector.scalar_tensor_tensor(
                        vdec[:, hw], in0=i2[:, hw], scalar=0.1,
                        in1=ps[hh][:], op0=A.mult, op1=A.add)

                # nm = (rho <= 0): not refractory  (Pool: no PSUM dep)
                nm = epool.tile([128, 2 * B_LOC], f32, name="nm")
                nc.gpsimd.tensor_scalar(nm[:], r2[:], 0.0, None, op0=A.is_le)
                # z_new = (vdec > 1) * nm   (bf16 0/1 is exact)
                nc.vector.scalar_tensor_tensor(
                    zo_t[:], in0=vdec[:], scalar=1.0, in1=nm[:],
                    op0=A.is_gt, op1=A.mult)
                # v_new = nm*((vdec<=1)*vdec - v) + v  (hold v while refrac)
                nc.vector.scalar_tensor_tensor(
                    vdec[:], in0=vdec[:], scalar=1.0, in1=vdec[:],
                    op0=A.is_le, op1=A.mult)
                nc.vector.tensor_sub(vdec[:], vdec[:], v2)
                nc.vector.tensor_mul(vdec[:], vdec[:], nm[:])
                nc.vector.tensor_add(vo_t[:], vdec[:], v2)
                # rho_new = r3 + z*(5 - r3),  r3 = relu(rho-1) = relu(rho-mask)
                r3 = epool.tile([128, 2 * B_LOC], f32, name="r3")
                nc.gpsimd.tensor_scalar(
                    r3[:], r2[:], 1.0, 0.0, op0=A.subtract, op1=A.max)
                t5 = epool.tile([128, 2 * B_LOC], f32, name="t5")
                nc.gpsimd.tensor_scalar(
                    t5[:], r3[:], 5.0, None, op0=A.subtract)
                nc.gpsimd.tensor_mul(t5[:], t5[:], zo_t[:])
                nc.gpsimd.tensor_sub(ro_t[:], r3[:], t5[:])

                # i_new GEMM: k<16 inp-part, k>=16 z-part (hp0: done above)
                if hp != 0:
                    ps2s = []
                    ps2zs = []
                    for hh in range(2):
                        ps2 = pspool.tile([128, B_LOC], f32, name="ps2")
                        mm2x(ps2, wcs[hh][1], range(KT1))
                        ps2z = pspool.tile([128, B_LOC], f32, name="ps2z")
                        mm2z(ps2z, wcs[hh][1], range(KT1))
                        ps2s.append(ps2)
                        ps2zs.append(ps2z)

                # i_new = 0.8*i + ps2  (DVE: gpsimd cannot read PSUM)
                for hh in range(2):
                    hw = slice(hh * B_LOC, (hh + 1) * B_LOC)
                    nc.vector.scalar_tensor_tensor(
                        io_t[:, hw], in0=i2[:, hw], scalar=0.8,
                        in1=ps2s[hh][:], op0=A.mult, op1=A.add)
                    nc.vector.tensor_add(
                        io_t[:, hw], io_t[:, hw], ps2zs[hh][:])

                # split the store by readiness: [z|v] first, then [rho]
                # (early), then [i] per half — the final store is only the
                # odd i half, so the kernel tail after the last matmul is
                # one DVE add plus a 0.25 MB DMA
                ob = hp * 8 * B_LOC
                nc.scalar.dma_start(
                    out_d[:, ob:ob + 4 * B_LOC], ot[:, :4 * B_LOC])
                nc.scalar.dma_start(
                    out_d[:, ob + 6 * B_LOC:ob + 8 * B_LOC],
                    ot[:, 6 * B_LOC:])
                nc.scalar.dma_start(
                    out_d[:, ob + 4 * B_LOC:ob + 5 * B_LOC],
                    ot[:, 4 * B_LOC:5 * B_LOC])
                nc.scalar.dma_start(
                    out_d[:, ob + 5 * B_LOC:ob + 6 * B_LOC],
                    ot[:, 5 * B_LOC:6 * B_LOC])

    nc.compile()
    return nc


def _sw_act(x, dt=np.float32):
    """[B_LOC, K] -> [128, KT*B_LOC] with layout [p, kt, b]."""
    a = np.ascontiguousarray(x.T).reshape(KT1, 128, B_LOC).transpose(1, 0, 2)
    return np.ascontiguousarray(a).reshape(128, KT1 * B_LOC).astype(dt)


def _sw_w(WT, kt, dt=np.float32):
    """WT=[K,H] -> [128, HT*kt*128] with layout [p, ht, kt, c]."""
    a = WT.reshape(kt, 128, HT, 128)              # [k, p, h, c]
    return np.ascontiguousarray(
        a.transpose(1, 2, 0, 3)).reshape(128, HT * kt * 128).astype(dt)


def kernel(inp, z, v, i, rho, input_weights, recurrent_weights, g_coupling):
    inp = np.ascontiguousarray(inp, dtype=np.float32)
    z = np.ascontiguousarray(z, dtype=np.float32)
    v = np.ascontiguousarray(v, dtype=np.float32)
    i = np.ascontiguousarray(i, dtype=np.float32)
    rho = np.ascontiguousarray(rho, dtype=np.float32)

    if "nc" not in _cache:
        _cache["nc"] = build()
    nc = _cache["nc"]
    wkey = (id(input_weights), id(recurrent_weights), id(g_coupling))
    if _cache.get("wkey") != wkey:
        G = np.ascontiguousarray(
            np.asarray(g_coupling, np.float32).T
            + np.float32(0.9) * np.eye(H, dtype=np.float32))
        Wi = np.ascontiguousarray(np.asarray(input_weights, np.float32).T)
        Wr = np.ascontiguousarray(np.asarray(recurrent_weights, np.float32).T)
        _cache["w"] = (_sw_w(G, KT1), _sw_w(Wi, KT1, BF16),
                       _sw_w(Wr, KT1, FP8))
        _cache["wkey"] = wkey
    w1, w2, w3 = _cache["w"]

    in_maps = []
    for c in range(NCORES):
        s = slice(c * B_LOC, (c + 1) * B_LOC)
        it = _sw_act(i[s], BF16).reshape(128, HP, 2 * B_LOC)
        rt = _sw_act(rho[s], BF16).reshape(128, HP, 2 * B_LOC)
        st = np.ascontiguousarray(
            np.concatenate([it, rt], axis=2)).reshape(128, -1)
        in_maps.append({
            "vt": _sw_act(v[s]),
            "zt": _sw_act(z[s], FP8), "xt": _sw_act(inp[s], BF16),
            "st": st, "w1": w1, "w2": w2, "w3": w3,
        })

    res = bass_utils.run_bass_kernel_spmd(
        nc, in_maps, core_ids=list(range(NCORES)),
        trace=bool(int(os.environ.get("LIF_TRACE", "0"))),
    )
    _cache["last_results"] = res

    outs = []
    for name in range(4):
        outs.append(np.empty((B, H), np.float32))
    for c in range(NCORES):
        o = np.asarray(res.results[c]["out"]).astype(np.float32)
        o = o.reshape(128, HP, 4, 2 * B_LOC)
        for j in range(4):
            a = o[:, :, j, :].reshape(128, HT, B_LOC)
            a = a.transpose(1, 0, 2).reshape(H, B_LOC)
            outs[j][c * B_LOC:(c + 1) * B_LOC] = a.T
    return np.stack(outs)
